# revision 12
# baseline (speedup 1.0000x reference)
"""Bilateral-grid slice (trilinear grid sample + per-pixel affine) on 8 TRN2
NeuronCores.

Strategy: data-parallel over views (N=16 -> 2 views/core). Host precomputes a
corner-duplicated cube table per view: one 256B bf16 row per grid cell holding
the 8 trilinear corner values for all 12 channels. Per core, per view:
  1. DVE scalar pipeline computes coords/fracs/weights and the flat cell index
     per pixel; 8 PE matmuls with 0/1 selection matrices fold the [128, T]
     cell-index tile into the SWDGE wrapped index layout (queue q's indices
     live in partitions 32q..32q+31); ACT engine copies PSUM -> int16.
  2. Pixel columns are processed in groups of 32: four dma_gather calls (8
     cols x 128 rows each, one per SWDGE queue, concurrent) stream bf16 cube
     rows from HBM into a pixel-major SBUF tile.
  3. DVE blends in bf16 (weights x corners, reduce to the 3x4 affine) and
     applies the per-pixel affine in f32.

Self-contained: expects full unsharded inputs, returns the full output.
"""
from contextlib import ExitStack

import numpy as np

import concourse.bass as bass
import concourse.bacc as bacc
import concourse.tile as tile
import concourse.mybir as mybir
from concourse.masks import make_identity

F32 = mybir.dt.float32
BF16 = mybir.dt.bfloat16
I16 = mybir.dt.int16
AL = mybir.AluOpType
AX = mybir.AxisListType

GRID_CELLS = 2048  # 8z * 16y * 16x
N_CORES = 8
GRAY_W = (0.299, 0.587, 0.114)

T = 448            # pixel columns per chunk
GROUP = 64         # columns per blend group
COLS = 8           # columns per dma_gather call (ring limit at default scratch)
NQ = 4


def _mkap(t_ap, p0, pn, free, off_elems):
    pitch = t_ap.ap[0][0]
    return bass.AP(
        tensor=t_ap.tensor,
        offset=t_ap.offset + p0 * pitch + off_elems,
        ap=[[pitch, pn]] + [[s, c] for (s, c) in free],
    )


def _bsel_matrices():
    b = np.zeros((128, 8, 128), dtype=np.float32)
    for h in range(8):
        for m in range(128):
            b[16 * h + (m % 16), h, m] = 1.0
    return b


def _cube_table(grids):
    """grids (N, 12, 8, 16, 16) f32 -> corner-duplicated (N, 2048, 128) bf16.

    Row for cell (z0,y0,x0): elem c*8 + j = grids[c, z0+dz, y0+dy, x0+dx],
    j = dz*4 + dy*2 + dx. Boundary cells (z0>6 | y0>14 | x0>14) are never
    referenced (coords are clamped) and stay zero.
    """
    import ml_dtypes
    N = grids.shape[0]
    g = grids.reshape(N, 12, 8, 16, 16)
    cube = np.zeros((N, 8, 16, 16, 12, 8), dtype=np.float32)
    for j, (dz, dy, dx) in enumerate(
            (dz, dy, dx) for dz in (0, 1) for dy in (0, 1) for dx in (0, 1)):
        cube[:, :7, :15, :15, :, j] = np.transpose(
            g[:, :, dz:dz + 7, dy:dy + 15, dx:dx + 15], (0, 2, 3, 4, 1))
    cube = cube.reshape(N, GRID_CELLS, 96)
    out = np.zeros((N, GRID_CELLS, 128), dtype=ml_dtypes.bfloat16)
    out[:, :, :96] = cube.astype(ml_dtypes.bfloat16)
    return out


def _build_core_kernel(n_views, Npx, scratch=81920, num_devices=8,
                       T=T, GROUP=GROUP, gat_bufs=2, bln_bufs=1,
                       skip_blend=0, skip_gather=0, aff_engine=1):
    assert Npx % 128 == 0
    B = Npx // 128

    nc = bacc.Bacc("TRN2", debug=False, num_devices=num_devices,
                   dynamic_dma_scratch_size=scratch, num_swdge_queues=NQ)
    cube_d = nc.dram_tensor("cube", [n_views, GRID_CELLS, 128], BF16,
                            kind="ExternalInput").ap()
    gxy_d = nc.dram_tensor("gxy", [n_views, Npx, 2], F32, kind="ExternalInput").ap()
    rgb_d = nc.dram_tensor("rgb", [n_views, Npx, 3], F32, kind="ExternalInput").ap()
    bsel_d = nc.dram_tensor("bsel", [128, 8, 128], F32, kind="ExternalInput").ap()
    out_d = nc.dram_tensor("out", [n_views, Npx, 3], F32, kind="ExternalOutput").ap()

    with tile.TileContext(nc) as tc, ExitStack() as ctx:
        consts = ctx.enter_context(tc.tile_pool(name="consts", bufs=1))
        inp = ctx.enter_context(tc.tile_pool(name="inp", bufs=2))
        scal = ctx.enter_context(tc.tile_pool(name="scal", bufs=1))
        wp = ctx.enter_context(tc.tile_pool(name="wp", bufs=2))
        idxp = ctx.enter_context(tc.tile_pool(name="idxp", bufs=2))
        psF = ctx.enter_context(tc.tile_pool(name="psF", bufs=4, space="PSUM"))
        gat = ctx.enter_context(tc.tile_pool(name="gat", bufs=gat_bufs))
        bln = ctx.enter_context(tc.tile_pool(name="bln", bufs=bln_bufs))
        outp = ctx.enter_context(tc.tile_pool(name="outp", bufs=2))

        ident = consts.tile([128, 128], F32)
        make_identity(nc, ident)
        bsel_sb = consts.tile([128, 8, 128], F32)
        nc.sync.dma_start(out=bsel_sb, in_=bsel_d)
        ones = consts.tile([128, T], F32)  # scal pool trimmed below
        nc.vector.memset(ones, 1.0)

        nT = (B + T - 1) // T

        for v in range(n_views):
            for ci in range(nT):
                t0 = ci * T
                Tc = min(T, B - t0)
                # queue j owns column blocks b (8 cols) with b % NQ == j
                nblk = (Tc + COLS - 1) // COLS
                qblocks = [[b for b in range(nblk) if b % NQ == j] for j in range(NQ)]

                gxy_t = inp.tile([128, T, 2], F32, tag="gxy", name="gxyt")
                rgb_t = inp.tile([128, T, 3], F32, tag="rgb", name="rgbt")
                gxy_v = gxy_d[v].rearrange("(p b) k -> p b k", p=128)
                rgb_v = rgb_d[v].rearrange("(p b) k -> p b k", p=128)
                nc.sync.dma_start(out=gxy_t[:, :Tc], in_=gxy_v[:, t0:t0 + Tc])
                nc.sync.dma_start(out=rgb_t[:, :Tc], in_=rgb_v[:, t0:t0 + Tc])

                def st(tag):
                    return scal.tile([128, T], F32, tag=tag, name=tag)

                x_t, y_t, z_t = st("x"), st("y"), st("z")
                nc.vector.tensor_scalar_mul(x_t[:, :Tc], gxy_t[:, :Tc, 0], 15.0)
                nc.vector.tensor_scalar_mul(y_t[:, :Tc], gxy_t[:, :Tc, 1], 15.0)
                nc.vector.tensor_scalar_mul(z_t[:, :Tc], rgb_t[:, :Tc, 0], GRAY_W[0] * 7)
                nc.vector.scalar_tensor_tensor(
                    z_t[:, :Tc], rgb_t[:, :Tc, 1], GRAY_W[1] * 7, z_t[:, :Tc],
                    AL.mult, AL.add)
                nc.vector.scalar_tensor_tensor(
                    z_t[:, :Tc], rgb_t[:, :Tc, 2], GRAY_W[2] * 7, z_t[:, :Tc],
                    AL.mult, AL.add)

                f_t, c0_t = {}, {}
                ii_t = scal.tile([128, T], mybir.dt.int32, tag="ii", name="ii")
                for nm, src, hi in (("x", x_t, 14.0), ("y", y_t, 14.0), ("z", z_t, 6.0)):
                    nc.scalar.copy(out=ii_t[:, :Tc], in_=src[:, :Tc])
                    c0 = st(nm + "0")
                    nc.scalar.copy(out=c0[:, :Tc], in_=ii_t[:, :Tc])
                    fr = st("fr")
                    nc.vector.tensor_tensor(fr[:, :Tc], c0[:, :Tc], src[:, :Tc], AL.is_gt)
                    nc.vector.tensor_tensor(c0[:, :Tc], c0[:, :Tc], fr[:, :Tc], AL.subtract)
                    nc.vector.tensor_scalar_min(c0[:, :Tc], c0[:, :Tc], hi)
                    f = st("f" + nm)
                    nc.vector.tensor_tensor(f[:, :Tc], src[:, :Tc], c0[:, :Tc], AL.subtract)
                    f_t[nm], c0_t[nm] = f, c0

                cellf = st("cellf")
                nc.vector.scalar_tensor_tensor(
                    cellf[:, :Tc], c0_t["z"][:, :Tc], 16.0, c0_t["y"][:, :Tc],
                    AL.mult, AL.add)
                nc.vector.scalar_tensor_tensor(
                    cellf[:, :Tc], cellf[:, :Tc], 16.0, c0_t["x"][:, :Tc],
                    AL.mult, AL.add)

                omx, omy, omz = st("omx"), st("omy"), st("omz")
                nc.vector.tensor_tensor(omx[:, :Tc], ones[:, :Tc], f_t["x"][:, :Tc], AL.subtract)
                nc.vector.tensor_tensor(omy[:, :Tc], ones[:, :Tc], f_t["y"][:, :Tc], AL.subtract)
                nc.vector.tensor_tensor(omz[:, :Tc], ones[:, :Tc], f_t["z"][:, :Tc], AL.subtract)
                pyx = []
                pyx_tags = ["x", "y", "z", "fr"]  # dead temps, reuse their space
                for wy in (omy, f_t["y"]):
                    for wx in (omx, f_t["x"]):
                        p = scal.tile([128, T], F32, tag=pyx_tags[len(pyx)],
                                      name=f"pyx{len(pyx)}")
                        nc.vector.tensor_tensor(p[:, :Tc], wy[:, :Tc], wx[:, :Tc], AL.mult)
                        pyx.append(p)
                w8_t = wp.tile([128, T, 8], BF16, tag="w8", name="w8")
                for jj in range(8):
                    wz = omz if jj < 4 else f_t["z"]
                    nc.vector.tensor_tensor(
                        w8_t[:, :Tc, jj], wz[:, :Tc], pyx[jj % 4][:, :Tc], AL.mult)

                # ---- wrapped idx layout: queue j -> partitions 32j..32j+31 ----
                # queue-local column order = ascending absolute column; the
                # gather for block b reads free offset (local ordinal)*8.
                idx_all = idxp.tile([128, T // NQ + COLS, 8], I16, tag="idx",
                                    name="idxall")
                for h in range(8):
                    pf = psF.tile([128, T], F32, tag="psf", name="psf")
                    nc.tensor.matmul(pf[:, :Tc], bsel_sb[:, h, :], cellf[:, :Tc],
                                     start=True, stop=True)
                    for j in range(NQ):
                        # queue j's blocks are b = j, j+NQ, ... (COLS cols each);
                        # one strided copy covers all its full blocks.
                        full = [b for b in qblocks[j] if (b + 1) * COLS <= Tc]
                        if full:
                            nc.scalar.copy(
                                out=_mkap(idx_all[:], 32 * j, 32,
                                          [(COLS * 8, len(full)), (8, COLS)], h),
                                in_=_mkap(pf[:], 32 * j, 32,
                                          [(NQ * COLS, len(full)), (1, COLS)],
                                          full[0] * COLS),
                            )
                        for bi, b in enumerate(qblocks[j]):
                            if (b + 1) * COLS <= Tc:
                                continue
                            w = Tc - b * COLS
                            nc.scalar.copy(
                                out=_mkap(idx_all[:], 32 * j, 32, [(8, w)],
                                          (bi * COLS) * 8 + h),
                                in_=_mkap(pf[:], 32 * j, 32, [(1, w)], b * COLS),
                            )

                out_t = outp.tile([128, T, 3], F32, tag="out", name="outt")

                # ---- gather + blend in groups of GROUP columns ----
                ngrp = (Tc + GROUP - 1) // GROUP
                for g in range(ngrp):
                    g0 = g * GROUP
                    gc = min(GROUP, Tc - g0)
                    nb = (gc + COLS - 1) // COLS
                    if not skip_gather:
                        cube_m = gat.tile([128, GROUP, 128], BF16, tag="cm", name="cm")
                    for jb in range(nb):
                        b = g * (GROUP // COLS) + jb  # absolute block index
                        q = b % NQ
                        bi = b // NQ                  # queue-local ordinal
                        w = min(COLS, gc - jb * COLS)
                        if skip_gather:
                            continue
                        nc.gpsimd.dma_gather(
                            out_ap=cube_m[:, jb * COLS: jb * COLS + w, :],
                            in_ap=cube_d[v],
                            idxs_ap=_mkap(idx_all[:], 0, 128, [(1, w * 8)],
                                          (bi * COLS) * 8),
                            num_idxs=w * 128,
                            num_idxs_reg=w * 128,
                            elem_size=128,
                            queue_num=q,
                        )
                    if skip_blend:
                        nc.vector.tensor_copy(
                            out_t[:, g0:g0 + gc], w8_t[:, g0:g0 + gc, :3])
                        continue
                    S_t = bln.tile([128, GROUP, 12, 8], BF16, tag="S", name="St")
                    nc.vector.tensor_tensor(
                        S_t[:, :gc],
                        cube_m[:, :gc, :96].rearrange("p m (c j) -> p m c j", j=8),
                        _mkap(w8_t[:], 0, 128, [(8, gc), (0, 12), (1, 8)], g0 * 8),
                        AL.mult)
                    # pairwise add tree replaces tensor_reduce (reduce is ~3x
                    # slower per element on DVE)
                    r1 = bln.tile([128, GROUP, 12, 4], BF16, tag="r1", name="r1")
                    nc.vector.tensor_tensor(
                        r1[:, :gc], S_t[:, :gc, :, 0:4], S_t[:, :gc, :, 4:8], AL.add)
                    r2 = bln.tile([128, GROUP, 12, 2], BF16, tag="r2", name="r2")
                    nc.vector.tensor_tensor(
                        r2[:, :gc], r1[:, :gc, :, 0:2], r1[:, :gc, :, 2:4], AL.add)
                    aff = bln.tile([128, GROUP, 12], F32, tag="aff", name="aff")
                    nc.vector.tensor_tensor(
                        aff[:, :gc], r2[:, :gc, :, 0], r2[:, :gc, :, 1], AL.add)
                    eng = nc.gpsimd if aff_engine else nc.vector
                    S2 = bln.tile([128, GROUP, 3, 3], F32, tag="S2", name="S2")
                    eng.tensor_tensor(
                        S2[:, :gc],
                        _mkap(aff[:], 0, 128, [(12, gc), (4, 3), (1, 3)], 0),
                        _mkap(rgb_t[:], 0, 128, [(3, gc), (0, 3), (1, 3)], g0 * 3),
                        AL.mult)
                    t3 = bln.tile([128, GROUP, 3], F32, tag="t3", name="t3")
                    eng.tensor_tensor(
                        t3[:, :gc], S2[:, :gc, :, 0], S2[:, :gc, :, 1], AL.add)
                    eng.tensor_tensor(
                        t3[:, :gc], t3[:, :gc], S2[:, :gc, :, 2], AL.add)
                    eng.tensor_tensor(
                        out_t[:, g0:g0 + gc],
                        t3[:, :gc],
                        _mkap(aff[:], 0, 128, [(12, gc), (4, 3)], 3),
                        AL.add)

                out_v = out_d[v].rearrange("(p b) k -> p b k", p=128)
                nc.scalar.dma_start(out=out_v[:, t0:t0 + Tc], in_=out_t[:, :Tc])

    nc.finalize()
    return nc


# ---------------- PJRT runner ----------------

def _make_runner(nc, n_cores):
    import jax
    import jax.core
    from jax.sharding import Mesh, PartitionSpec
    from jax.experimental.shard_map import shard_map
    from concourse.bass2jax import _bass_exec_p, partition_id_tensor, install_neuronx_cc_hook

    install_neuronx_cc_hook()
    partition_name = nc.partition_id_tensor.name if nc.partition_id_tensor else None
    in_names, out_names, out_avals, zero_shapes = [], [], [], []
    for alloc in nc.m.functions[0].allocations:
        if not isinstance(alloc, mybir.MemoryLocationSet):
            continue
        name = alloc.memorylocations[0].name
        if alloc.kind == "ExternalInput":
            if name != partition_name:
                in_names.append(name)
        elif alloc.kind == "ExternalOutput":
            shape = tuple(alloc.tensor_shape)
            dtype = mybir.dt.np(alloc.dtype)
            out_names.append(name)
            out_avals.append(jax.core.ShapedArray(shape, dtype))
            zero_shapes.append((shape, dtype))

    n_params = len(in_names)
    n_outs = len(out_avals)
    all_in_names = list(in_names) + list(out_names)
    if partition_name is not None:
        all_in_names.append(partition_name)

    def _body(*args):
        operands = list(args)
        if partition_name is not None:
            operands.append(partition_id_tensor())
        return tuple(_bass_exec_p.bind(
            *operands,
            out_avals=tuple(out_avals),
            in_names=tuple(all_in_names),
            out_names=tuple(out_names),
            lowering_input_output_aliases=(),
            sim_require_finite=True,
            sim_require_nnan=True,
            nc=nc,
        ))

    donate = tuple(range(n_params, n_params + n_outs))
    devices = jax.devices()[:n_cores]
    mesh = Mesh(np.asarray(devices), ("core",))
    in_specs = (PartitionSpec("core"),) * (n_params + n_outs)
    out_specs = (PartitionSpec("core"),) * n_outs
    jf = jax.jit(
        shard_map(_body, mesh=mesh, in_specs=in_specs, out_specs=out_specs,
                  check_rep=False),
        donate_argnums=donate, keep_unused=True)

    def run(in_maps):
        concat_in = [
            np.concatenate([np.asarray(in_maps[c][n]) for c in range(n_cores)], axis=0)
            for n in in_names
        ]
        concat_zeros = [np.zeros((n_cores * s[0], *s[1:]), d) for (s, d) in zero_shapes]
        out_arrs = jf(*concat_in, *concat_zeros)
        jax.block_until_ready(out_arrs)
        return [
            {n: np.asarray(out_arrs[i]).reshape(n_cores, *out_avals[i].shape)[c]
             for i, n in enumerate(out_names)}
            for c in range(n_cores)
        ]

    return run, jf, in_names, zero_shapes


_CACHE = {}


def _get_runner(n_views, Npx, **kw):
    key = (n_views, Npx, tuple(sorted(kw.items())))
    if key not in _CACHE:
        nc = _build_core_kernel(n_views, Npx, **kw)
        _CACHE[key] = _make_runner(nc, N_CORES)
    return _CACHE[key]


def kernel(grids, grid_xy, rgb, idx):
    grids = np.ascontiguousarray(np.asarray(grids, dtype=np.float32))
    grid_xy = np.ascontiguousarray(np.asarray(grid_xy, dtype=np.float32))
    rgb = np.ascontiguousarray(np.asarray(rgb, dtype=np.float32))
    idx = np.asarray(idx)
    N, Hi, Wi, _ = rgb.shape
    Npx = Hi * Wi
    V = N // N_CORES
    g = np.take(grids, idx, axis=0)  # (N, 12, 8, 16, 16)
    cube = _cube_table(g)            # (N, 2048, 128) bf16

    run, _, _, _ = _get_runner(V, Npx)
    bsel = _bsel_matrices()
    in_maps = [{
        "cube": cube[c * V:(c + 1) * V],
        "gxy": grid_xy[c * V:(c + 1) * V].reshape(V, Npx, 2),
        "rgb": rgb[c * V:(c + 1) * V].reshape(V, Npx, 3),
        "bsel": bsel,
    } for c in range(N_CORES)]
    results = run(in_maps)
    out = np.stack([r["out"] for r in results], axis=0)  # (8, V, Npx, 3)
    return out.reshape(N, Hi, Wi, 3)
